# revision 9
# baseline (speedup 1.0000x reference)
"""Distributed Trainium2 Bass kernel for BitNet-style attention block.

Sharding: sequence-parallel projections + (batch x kv-head) parallel attention,
stitched with two AllToAll collectives. 8 cores.

Per core (core i):
  A. RMSNorm + per-token absmax int8-style quantization of its 512-token chunk
     (tokens i*256..(i+1)*256 of both batches).
  B. qkv projection as exact integer bf16 matmul against host-prequantized
     ternary weights, dequant, RoPE on q/k, scatter into AllToAll #1 send buf.
  C. AllToAll #1 -> core i holds full-sequence q (4 heads), k, v for kv-head i
     of both batches; causal attention (transposed scores, exp on ACT,
     ones-column rowsums, deferred normalization).
  D. AllToAll #2 -> core i holds its token chunk of all 32 heads; per-token
     quantization + integer matmul with ternary out-projection weights.
"""
import sys
sys.path.insert(0, "/opt/trn_rl_repo")
import numpy as np
import ml_dtypes
import concourse.bass as bass
import concourse.tile as tile
from concourse import bacc, mybir
from concourse import bass_utils

f32 = mybir.dt.float32
bf16 = mybir.dt.bfloat16
FT = mybir.ActivationFunctionType
ALU = mybir.AluOpType

B, S, H = 2, 2048, 2048
NH, NKV, HD = 32, 8, 64
G = NH // NKV                    # 4
QKV_O = (NH + 2 * NKV) * HD      # 3072
EPS = 1e-5
THETA = 10000.0
C = 8
SC = S // C                      # 256 positions per core
TOK = B * SC                     # 512 token rows per core
MAGIC = float(1.5 * 2.0 ** 23)   # rounds-to-nearest-even for |v| < 2^22, signed
NT = TOK // 128                  # 4 token tiles per core
NHT = H // 128                   # 16 h-tiles
NKT = S // 128                   # 16 kj tiles

# a2a1 column layout per dest: [b0q 256 | b1q 256 | b0k 64 | b1k 64 | b0v 64 | b1v 64]
A2A1_W = 2 * G * HD + 2 * HD + 2 * HD    # 768
COL_K = 512
COL_V = 640


def _dap(t_ap, extra, dims):
    return bass.AP(tensor=t_ap.tensor, offset=t_ap.offset + extra, ap=[list(d) for d in dims])


def build_nc():
    nc = bacc.Bacc("TRN2", target_bir_lowering=False, debug=False, num_devices=C)

    x_in = nc.dram_tensor("x", [TOK, H], f32, kind="ExternalInput")
    wn_in = nc.dram_tensor("wn", [1, H], f32, kind="ExternalInput")
    wq1t_in = nc.dram_tensor("wq1t", [H, QKV_O], bf16, kind="ExternalInput")
    wq2t_in = nc.dram_tensor("wq2t", [H, H], bf16, kind="ExternalInput")
    cos_in = nc.dram_tensor("cosb", [SC, 8 * 32], f32, kind="ExternalInput")
    sin_in = nc.dram_tensor("sinb", [SC, 8 * 32], f32, kind="ExternalInput")
    tri_in = nc.dram_tensor("trimask", [128, 128], bf16, kind="ExternalInput")
    sw1_in = nc.dram_tensor("sw1", [1, 1], f32, kind="ExternalInput")
    sw2_in = nc.dram_tensor("sw2", [1, 1], f32, kind="ExternalInput")
    out_ext = nc.dram_tensor("out", [TOK, H], f32, kind="ExternalOutput")

    X = x_in.ap()
    WQ1 = wq1t_in.ap()
    WQ2 = wq2t_in.ap()
    OUT = out_ext.ap()

    with tile.TileContext(nc) as tc:
        from contextlib import ExitStack
        with ExitStack() as top:
            dram = top.enter_context(tc.tile_pool(name="dram", bufs=1, space="DRAM"))
            const = top.enter_context(tc.tile_pool(name="const", bufs=1))
            smalls = top.enter_context(tc.tile_pool(name="smalls", bufs=1))
            psA = top.enter_context(tc.tile_pool(name="psA", bufs=4, space="PSUM"))
            psS = top.enter_context(tc.tile_pool(name="psS", bufs=2, space="PSUM"))

            # ---------------- DRAM scratch ----------------
            xq_d = dram.tile([TOK, H], bf16)
            a2a1_i = dram.tile([C * SC, A2A1_W], bf16)
            a2a1_o = dram.tile([C * SC, A2A1_W], bf16)
            a2a2_i = dram.tile([C * B * SC, G * HD], bf16)
            a2a2_o = dram.tile([C * B * SC, G * HD], bf16)
            xq2_d = dram.tile([TOK, H], bf16)

            # ---------------- constants ----------------
            wnorm_b = const.tile([128, H], f32)
            nc.sync.dma_start(out=wnorm_b[:], in_=_dap(wn_in.ap(), 0, [[0, 128], [1, H]]))
            trim = const.tile([128, 128], bf16)
            nc.sync.dma_start(out=trim[:], in_=tri_in.ap()[:, :])
            sw1b = const.tile([128, 1], f32)
            nc.sync.dma_start(out=sw1b[:], in_=_dap(sw1_in.ap(), 0, [[0, 128], [1, 1]]))
            sw2b = const.tile([128, 1], f32)
            nc.sync.dma_start(out=sw2b[:], in_=_dap(sw2_in.ap(), 0, [[0, 128], [1, 1]]))
            epsb = const.tile([128, 1], f32)
            nc.vector.memset(epsb[:], EPS)

            d1s = [smalls.tile([128, 1], f32, name=f"d1_{m}") for m in range(NT)]
            d2s = [smalls.tile([128, 1], f32, name=f"d2_{m}") for m in range(NT)]

            xqT_pool = top.enter_context(tc.tile_pool(name="xqT", bufs=NHT))

            # ================= Stage A: RMSNorm + quantize =================
            with ExitStack() as sa:
                pA = sa.enter_context(tc.tile_pool(name="pA", bufs=2))
                pSc = sa.enter_context(tc.tile_pool(name="pASc", bufs=4))
                for m in range(NT):
                    xa = pA.tile([128, H], f32, tag="xa")
                    nc.sync.dma_start(out=xa[:], in_=X[m * 128:(m + 1) * 128, :])
                    sq = pA.tile([128, H], f32, tag="sq")
                    ssq = pSc.tile([128, 1], f32, tag="ssq")
                    nc.scalar.activation(out=sq[:], in_=xa[:], func=FT.Square, accum_out=ssq[:])
                    xw = pA.tile([128, H], f32, tag="xw")
                    nc.vector.tensor_tensor(xw[:], xa[:], wnorm_b[:], ALU.mult)
                    std = pSc.tile([128, 1], f32, tag="std")
                    nc.scalar.activation(out=std[:], in_=ssq[:], func=FT.Sqrt,
                                         bias=epsb[:], scale=1.0 / H)
                    rstd = pSc.tile([128, 1], f32, tag="rstd")
                    nc.vector.reciprocal(rstd[:], std[:])
                    mx = pSc.tile([128, 1], f32, tag="mx")
                    nc.vector.tensor_reduce(mx[:], xw[:], mybir.AxisListType.X, ALU.max,
                                            apply_absolute_value=True)
                    mp = pSc.tile([128, 1], f32, tag="mp")
                    nc.vector.tensor_scalar(mp[:], mx[:], rstd[:], 1e-5, ALU.mult, ALU.max)
                    nc.vector.tensor_tensor(d1s[m][:], mp[:], sw1b[:], ALU.mult)
                    rmp = pSc.tile([128, 1], f32, tag="rmp")
                    nc.vector.reciprocal(rmp[:], mp[:])
                    csc = pSc.tile([128, 1], f32, tag="csc")
                    nc.vector.tensor_scalar(csc[:], rmp[:], rstd[:], 127.0, ALU.mult, ALU.mult)
                    t1 = pA.tile([128, H], f32, tag="t1")
                    nc.scalar.activation(out=t1[:], in_=xw[:], func=FT.Copy,
                                         bias=MAGIC, scale=csc[:])
                    xqm = pA.tile([128, H], bf16, tag="xqm")
                    nc.vector.tensor_scalar(xqm[:], t1[:], MAGIC, None, ALU.subtract)
                    nc.sync.dma_start(out=xq_d[m * 128:(m + 1) * 128, :], in_=xqm[:])

            # transposed activations for the qkv matmul
            xqT = []
            for j in range(NHT):
                t = xqT_pool.tile([128, TOK], bf16, name=f"xqT_{j}", tag="xqT")
                nc.sync.dma_start(out=t[:], in_=xq_d[:, j * 128:(j + 1) * 128], transpose=True)
                xqT.append(t)

            # ================= Stage B: qkv matmul + RoPE + scatter ========
            with ExitStack() as sb:
                pW = sb.enter_context(tc.tile_pool(name="pW", bufs=3))
                pQC = sb.enter_context(tc.tile_pool(name="pQC", bufs=3))
                pRT = sb.enter_context(tc.tile_pool(name="pRT", bufs=2))
                pSend = sb.enter_context(tc.tile_pool(name="pSend", bufs=NT))
                pCos = sb.enter_context(tc.tile_pool(name="pCos", bufs=1))

                cosr = []
                sinr = []
                for par in range(2):
                    ct = pCos.tile([128, 8 * 32], f32, name=f"cosr_{par}")
                    nc.sync.dma_start(out=ct[:], in_=cos_in.ap()[par * 128:(par + 1) * 128, :])
                    st_ = pCos.tile([128, 8 * 32], f32, name=f"sinr_{par}")
                    nc.sync.dma_start(out=st_[:], in_=sin_in.ap()[par * 128:(par + 1) * 128, :])
                    cosr.append(ct)
                    sinr.append(st_)

                sends = [pSend.tile([128, QKV_O], bf16, name=f"sends_{m}", tag="sends")
                         for m in range(NT)]

                NQC = QKV_O // 512   # 6 chunks of 512
                for ng in range(NQC):
                    psq = [psA.tile([128, 512], f32, tag="acc", name=f"qkvp_{ng}_{m}")
                           for m in range(NT)]
                    for j in range(NHT):
                        wt = pW.tile([128, 512], bf16, tag="w1")
                        nc.sync.dma_start(out=wt[:],
                                          in_=WQ1[j * 128:(j + 1) * 128, ng * 512:(ng + 1) * 512])
                        for m in range(NT):
                            nc.tensor.matmul(psq[m][:], xqT[j][:, m * 128:(m + 1) * 128], wt[:],
                                             start=(j == 0), stop=(j == NHT - 1))
                    for m in range(NT):
                        par = m % 2
                        if ng < 5:
                            # dequant then rope (q chunks 0..3 hold 8 q-heads each; chunk 4 = k)
                            qc_t = pQC.tile([128, 512], f32, tag="qc")
                            nc.vector.tensor_scalar(qc_t[:], psq[m][:], d1s[m][:], None, ALU.mult)
                            xv = qc_t[:].rearrange("p (h t d) -> p h t d", t=2, d=32)
                            xr = xv[:, :, 0, :]
                            xi = xv[:, :, 1, :]
                            cv = cosr[par][:].rearrange("p (h d) -> p h d", d=32)
                            sv = sinr[par][:].rearrange("p (h d) -> p h d", d=32)
                            ov = sends[m][:, ng * 512:(ng + 1) * 512].rearrange(
                                "p (h t d) -> p h t d", t=2, d=32)
                            o_r = ov[:, :, 0, :]
                            o_i = ov[:, :, 1, :]
                            ta = pRT.tile([128, 256], f32, tag="ta")
                            tb = pRT.tile([128, 256], f32, tag="tb")
                            tav = ta[:].rearrange("p (h d) -> p h d", d=32)
                            tbv = tb[:].rearrange("p (h d) -> p h d", d=32)
                            nc.vector.tensor_tensor(tav, xr, cv, ALU.mult)
                            nc.vector.tensor_tensor(tbv, xi, sv, ALU.mult)
                            nc.vector.tensor_tensor(o_r, tav, tbv, ALU.subtract)
                            nc.vector.tensor_tensor(tav, xr, sv, ALU.mult)
                            nc.vector.tensor_tensor(tbv, xi, cv, ALU.mult)
                            nc.vector.tensor_tensor(o_i, tav, tbv, ALU.add)
                        else:
                            # v chunk: dequant straight to bf16 send cols
                            nc.vector.tensor_scalar(sends[m][:, ng * 512:(ng + 1) * 512],
                                                    psq[m][:], d1s[m][:], None, ALU.mult)

                # scatter sends into a2a1 input buffer
                a1i = a2a1_i[:]
                for m in range(NT):
                    b = m // 2
                    par = m % 2
                    base = par * 128 * A2A1_W
                    # q: 8 dests x 256 cols
                    nc.sync.dma_start(
                        out=_dap(a1i, base + b * 256,
                                 [[A2A1_W, 128], [SC * A2A1_W, 8], [1, 256]]),
                        in_=sends[m][:, 0:2048].rearrange("p (j c) -> p j c", j=8))
                    # k: 8 dests x 64 cols
                    nc.sync.dma_start(
                        out=_dap(a1i, base + COL_K + b * 64,
                                 [[A2A1_W, 128], [SC * A2A1_W, 8], [1, 64]]),
                        in_=sends[m][:, 2048:2560].rearrange("p (j c) -> p j c", j=8))
                    # v
                    nc.sync.dma_start(
                        out=_dap(a1i, base + COL_V + b * 64,
                                 [[A2A1_W, 128], [SC * A2A1_W, 8], [1, 64]]),
                        in_=sends[m][:, 2560:3072].rearrange("p (j c) -> p j c", j=8))

            nc.gpsimd.collective_compute(
                "AllToAll", ALU.bypass, replica_groups=[list(range(C))],
                ins=[a2a1_i[:].opt()], outs=[a2a1_o[:].opt()])

            # ================= Stage C: attention =========================
            with ExitStack() as sc:
                pQT = sc.enter_context(tc.tile_pool(name="pQT", bufs=4))
                pKT = sc.enter_context(tc.tile_pool(name="pKT", bufs=2))
                pVA = sc.enter_context(tc.tile_pool(name="pVA", bufs=NKT))
                pEX = sc.enter_context(tc.tile_pool(name="pEX", bufs=NKT))
                pOB = sc.enter_context(tc.tile_pool(name="pOB", bufs=2 * NKT))
                pR = sc.enter_context(tc.tile_pool(name="pR", bufs=8))

                qT = []
                for xch in range(4):
                    t = pQT.tile([128, S], bf16, name=f"qT_{xch}", tag="qT")
                    nc.sync.dma_start(out=t[:], in_=a2a1_o[:, xch * 128:(xch + 1) * 128],
                                      transpose=True)
                    qT.append(t)
                kT0 = pKT.tile([128, S], bf16, name="kT0", tag="kT")
                nc.sync.dma_start(out=kT0[:], in_=a2a1_o[:, COL_K:COL_K + 128], transpose=True)
                kSw = pKT.tile([128, S], bf16, name="kSw", tag="kT")
                nc.sync.dma_start(out=kSw[0:64, :], in_=kT0[64:128, :])
                nc.sync.dma_start(out=kSw[64:128, :], in_=kT0[0:64, :])

                va = []
                for kt in range(NKT):
                    t = pVA.tile([128, 130], bf16, name=f"va_{kt}", tag="va")
                    nc.sync.dma_start(out=t[:, 0:64],
                                      in_=a2a1_o[kt * 128:(kt + 1) * 128, COL_V:COL_V + 64])
                    nc.sync.dma_start(out=t[:, 65:129],
                                      in_=a2a1_o[kt * 128:(kt + 1) * 128, COL_V + 64:COL_V + 128])
                    nc.vector.memset(t[:, 64:65], 1.0)
                    nc.vector.memset(t[:, 129:130], 1.0)
                    va.append(t)

                obs = [[pOB.tile([128, G * HD], bf16, name=f"ob_{b}_{qt}", tag="ob")
                        for qt in range(NKT)] for b in range(B)]

                for b in range(B):
                    for hp in range(2):
                        qTx = qT[2 * b + hp]
                        lhs_ev = kT0 if b == 0 else kSw
                        lhs_od = kSw if b == 0 else kT0
                        for qc in range(4):
                            exs = []
                            for kt in range(4 * qc + 4):
                                dpos = max(0, kt * 128 - qc * 512)
                                st = psS.tile([128, 1024], f32, tag="st",
                                              name=f"st_{b}_{hp}_{qc}_{kt}")
                                nc.tensor.matmul(
                                    st[:, dpos:512],
                                    lhs_ev[0:64, kt * 128:(kt + 1) * 128],
                                    qTx[0:64, qc * 512 + dpos:(qc + 1) * 512],
                                    start=True, stop=True)
                                nc.tensor.matmul(
                                    st[:, 512 + dpos:1024],
                                    lhs_od[64:128, kt * 128:(kt + 1) * 128],
                                    qTx[64:128, qc * 512 + dpos:(qc + 1) * 512],
                                    start=True, stop=True, tile_position=(64, 0))
                                ex = pEX.tile([128, 1024], bf16, tag="ex",
                                              name=f"ex_{b}_{hp}_{qc}_{kt}")
                                n = 512 - dpos
                                stv = st[:].rearrange("p (h q) -> p h q", h=2)[:, :, dpos:512]
                                exv = ex[:].rearrange("p (h q) -> p h q", h=2)[:, :, dpos:512]
                                nc.scalar.activation(out=exv, in_=stv, func=FT.Exp, scale=0.125)
                                if kt >= 4 * qc:
                                    for h in range(2):
                                        sl = ex[:, h * 512 + dpos:h * 512 + dpos + 128]
                                        nc.vector.tensor_tensor(sl, sl, trim[:], ALU.mult)
                                exs.append(ex)
                            for h in range(2):
                                for qtl in range(4):
                                    qt = 4 * qc + qtl
                                    op = psA.tile([128, 65], f32, tag="acc",
                                                  name=f"op_{b}_{hp}_{qc}_{h}_{qtl}")
                                    for kt in range(qt + 1):
                                        nc.tensor.matmul(
                                            op[:],
                                            exs[kt][:, h * 512 + qtl * 128:h * 512 + (qtl + 1) * 128],
                                            va[kt][:, b * 65:(b + 1) * 65],
                                            start=(kt == 0), stop=(kt == qt))
                                    r = pR.tile([128, 1], f32, tag="r")
                                    nc.vector.reciprocal(r[:], op[:, 64:65])
                                    hg = hp * 2 + h
                                    nc.vector.tensor_scalar(
                                        obs[b][qt][:, hg * 64:(hg + 1) * 64],
                                        op[:, 0:64], r[:], None, ALU.mult)
                    # ship this batch's outputs
                    for qt in range(NKT):
                        j = qt // 2
                        rowbase = j * (B * SC) + b * SC + (qt % 2) * 128
                        nc.sync.dma_start(out=a2a2_i[rowbase:rowbase + 128, :], in_=obs[b][qt][:])

            nc.gpsimd.collective_compute(
                "AllToAll", ALU.bypass, replica_groups=[list(range(C))],
                ins=[a2a2_i[:].opt()], outs=[a2a2_o[:].opt()])

            # ================= Stage D: out projection ====================
            with ExitStack() as sd:
                pD = sd.enter_context(tc.tile_pool(name="pD", bufs=2))
                pDs = sd.enter_context(tc.tile_pool(name="pDs", bufs=4))
                pXT2 = sd.enter_context(tc.tile_pool(name="pXT2", bufs=NHT))
                pW2 = sd.enter_context(tc.tile_pool(name="pW2", bufs=3))
                pO = sd.enter_context(tc.tile_pool(name="pO", bufs=3))

                a2o = a2a2_o[:]
                for m in range(NT):
                    b = m // 2
                    r0 = (m % 2) * 128
                    x2 = pD.tile([128, H], bf16, tag="x2")
                    nc.sync.dma_start(
                        out=x2[:],
                        in_=_dap(a2o, (b * SC + r0) * 256,
                                 [[256, 128], [B * SC * 256, 8], [1, 256]]))
                    mx2 = pDs.tile([128, 1], f32, tag="mx2")
                    nc.vector.tensor_reduce(mx2[:], x2[:], mybir.AxisListType.X, ALU.max,
                                            apply_absolute_value=True)
                    mp2 = pDs.tile([128, 1], f32, tag="mp2")
                    nc.vector.tensor_scalar(mp2[:], mx2[:], 1e-5, None, ALU.max)
                    nc.vector.tensor_tensor(d2s[m][:], mp2[:], sw2b[:], ALU.mult)
                    rm2 = pDs.tile([128, 1], f32, tag="rm2")
                    nc.vector.reciprocal(rm2[:], mp2[:])
                    c2 = pDs.tile([128, 1], f32, tag="c2")
                    nc.vector.tensor_scalar(c2[:], rm2[:], 127.0, None, ALU.mult)
                    t2 = pD.tile([128, H], f32, tag="t2")
                    nc.scalar.activation(out=t2[:], in_=x2[:], func=FT.Copy,
                                         bias=MAGIC, scale=c2[:])
                    xq2 = pD.tile([128, H], bf16, tag="xq2")
                    nc.vector.tensor_scalar(xq2[:], t2[:], MAGIC, None, ALU.subtract)
                    nc.sync.dma_start(out=xq2_d[m * 128:(m + 1) * 128, :], in_=xq2[:])

                xq2T = []
                for j in range(NHT):
                    t = pXT2.tile([128, TOK], bf16, name=f"xq2T_{j}", tag="xq2T")
                    nc.sync.dma_start(out=t[:], in_=xq2_d[:, j * 128:(j + 1) * 128],
                                      transpose=True)
                    xq2T.append(t)

                for ng in range(H // 512):
                    ps2 = [psA.tile([128, 512], f32, tag="acc", name=f"ps2_{ng}_{m}")
                           for m in range(NT)]
                    for j in range(NHT):
                        wt = pW2.tile([128, 512], bf16, tag="w2")
                        nc.sync.dma_start(out=wt[:],
                                          in_=WQ2[j * 128:(j + 1) * 128, ng * 512:(ng + 1) * 512])
                        for m in range(NT):
                            nc.tensor.matmul(ps2[m][:], xq2T[j][:, m * 128:(m + 1) * 128], wt[:],
                                             start=(j == 0), stop=(j == NHT - 1))
                    for m in range(NT):
                        ot = pO.tile([128, 512], f32, tag="ot")
                        nc.vector.tensor_scalar(ot[:], ps2[m][:], d2s[m][:], None, ALU.mult)
                        nc.sync.dma_start(
                            out=OUT[m * 128:(m + 1) * 128, ng * 512:(ng + 1) * 512], in_=ot[:])

    nc.compile()
    return nc


_NC_CACHE = {}


def _get_nc():
    if "nc" not in _NC_CACHE:
        _NC_CACHE["nc"] = build_nc()
    return _NC_CACHE["nc"]


def kernel(x, w_norm, w_qkv, w_out):
    x = np.asarray(x, dtype=np.float32)
    w_norm = np.asarray(w_norm, dtype=np.float32)
    w_qkv = np.asarray(w_qkv, dtype=np.float32)
    w_out = np.asarray(w_out, dtype=np.float32)

    # host-side ternary weight quantization (matches reference bit_linear fwd)
    def tern(w):
        ws = np.float32(1.0) / np.clip(np.mean(np.abs(w)), np.float32(1e-5), None).astype(np.float32)
        wq = np.clip(np.round(w * ws), -1.0, 1.0).astype(np.float32)
        s = (np.float32(1.0) / ws).astype(np.float32)
        return wq, s

    wq1, s_w1 = tern(w_qkv)
    wq2, s_w2 = tern(w_out)
    wq1t = np.ascontiguousarray(wq1.T).astype(ml_dtypes.bfloat16)   # [H, 3072]
    wq2t = np.ascontiguousarray(wq2.T).astype(ml_dtypes.bfloat16)   # [H, H]

    inv_freq = (1.0 / THETA ** (np.arange(0, HD, 2, dtype=np.float32) / HD)).astype(np.float32)
    t_pos = np.arange(S, dtype=np.float32)
    freqs = t_pos[:, None] * inv_freq[None, :]
    cos_full = np.cos(freqs).astype(np.float32)
    sin_full = np.sin(freqs).astype(np.float32)

    trimask = np.triu(np.ones((128, 128), np.float32)).astype(ml_dtypes.bfloat16)
    sw1 = np.array([[s_w1 / np.float32(127.0)]], dtype=np.float32)
    sw2 = np.array([[s_w2 / np.float32(127.0)]], dtype=np.float32)
    wn2d = w_norm.reshape(1, H)

    in_maps = []
    for i in range(C):
        xc = np.ascontiguousarray(
            np.concatenate([x[0, i * SC:(i + 1) * SC, :], x[1, i * SC:(i + 1) * SC, :]], axis=0))
        in_maps.append({
            "x": xc,
            "wn": wn2d,
            "wq1t": wq1t,
            "wq2t": wq2t,
            "cosb": np.ascontiguousarray(np.tile(cos_full[i * SC:(i + 1) * SC, :], (1, 8))),
            "sinb": np.ascontiguousarray(np.tile(sin_full[i * SC:(i + 1) * SC, :], (1, 8))),
            "trimask": trimask,
            "sw1": sw1,
            "sw2": sw2,
        })

    nc = _get_nc()
    res = bass_utils.run_bass_kernel_spmd(nc, in_maps, core_ids=list(range(C)))

    out = np.empty((B, S, H), dtype=np.float32)
    for i in range(C):
        ci = res.results[i]["out"]
        for b in range(B):
            out[b, i * SC:(i + 1) * SC, :] = ci[b * SC:(b + 1) * SC, :]
    return out
